# revision 10
# baseline (speedup 1.0000x reference)
"""Trainium2 Bass kernel for causal multi-head attention.

Problem: B=4, S=2048, D=512, H=8 heads (head_dim 64), causal mask.
  q = x @ Wq.T + bq ; k = x @ Wk.T + bk ; v = x @ Wv.T + bv
  att = softmax(mask(q k^T / sqrt(64))) @ v ; out = att @ Wo.T + bo

Sharding: 8 cores = (batch b in 0..3) x (head-group hg in 0..1, 4 heads each).
Each core computes its 4 heads' Q/K/V projections, attention, and a partial
out-projection (contribution of its head block). Host sums the two partials
per batch and adds bo. No collectives needed.

v2 design notes (the previous version ran projections and attention as two
serial phases; engine queues execute in program order, so the Scalar engine
sat idle for the first ~40us and the whole kernel was exp-starved):

 - Projections are emitted PER S-BLOCK and interleaved into the attention
   unit stream of the previous q-block, so the PE queue alternates
   projection and attention matmuls and the exp stream starts almost
   immediately.
 - Attention is emitted in "units" of one k-tile x one head-pair. Each unit
   owns a [128, 1024] PSUM tile: head A's scores right-aligned in bank 0
   (cols 512-w..512), head B's in bank 1 (cols 512..512+w) - one contiguous
   exp over exactly 2w columns, no wasted Scalar cycles, and each matmul
   stays inside one PSUM bank. stU is double-buffered so scores(u+1) never
   waits on exp(u).
 - Scores are computed TRANSPOSED (ST[k, q]); exp(ST) is directly the
   moving operand of the attention*V matmul. The softmax denominator falls
   out of the same matmul via a ones-column per head in V. The ones column
   is FIRST in each head's 96-wide V segment (values 32-aligned), so sumexp
   lands on PSUM partition 0
   and 1/sumexp is one reciprocal_approx_fast + one partition_broadcast
   (no 32x32 transpose dance).
 - Head pairs use PE row-groups (rows 0..63 / 64..127, tile_position) so the
   two 64-contraction score matmuls execute concurrently.
 - causal structure exploited exactly: k-tiles above the diagonal skipped,
   band k-tiles only produce their valid q columns, one static [128,128]
   0/1 tile masks the leading triangle.
 - matmul operands bf16, accumulation fp32 in PSUM; output DMA'd as bf16
   (host sums partials in fp32 and adds bo).
 - bk is softmax-invariant (adds a per-query constant) and is not used.

The mask input is verified on the host: if it is exactly the causal mask the
fast path runs; otherwise a generic variant runs that reads a host-prepared
transposed multiplicative mask from DRAM.
"""

import sys

import numpy as np

for _p in ("/opt/trn_rl_repo",):
    if _p not in sys.path:
        sys.path.insert(0, _p)

import ml_dtypes  # noqa: E402

import concourse.bass as bass  # noqa: E402
import concourse.tile as tile  # noqa: E402
from concourse import bacc, mybir  # noqa: E402

B, S, D, H = 4, 2048, 512, 8
HD = D // H  # 64
P = 128
HG = 4  # heads per core
DG = HG * HD  # 256 per-core head dims
QB = 512  # q-block
NQB = S // QB  # 4
NKT = S // P  # 16 k-tiles
NET = D // P  # 4 contraction tiles for projections
VW = HG * HD  # 256: V-projection computes values only; the per-head ones
#   column (softmax denominator) is a memset constant.
VSEG = 128  # per-head V segment: [ones, 63 pad, 64 values] so sumexp lands
#   on partition 0 and values on partitions 64..127 (APs may not cross the
#   partition-64 half boundary mid-pattern, so bases must be 0 or 64)

F32 = mybir.dt.float32
BF16 = mybir.dt.bfloat16
NPBF16 = ml_dtypes.bfloat16

_BUILT = {}


def _build_nc(causal: bool):
    """Build (and bacc-compile) the SPMD single-core program."""
    nc = bacc.Bacc("TRN2", target_bir_lowering=False, debug=False, num_devices=8)

    xT_d = nc.dram_tensor("xT", [D, S], BF16, kind="ExternalInput").ap()
    wq_d = nc.dram_tensor("wq", [D, DG], BF16, kind="ExternalInput").ap()
    bq_d = nc.dram_tensor("bqv", [DG, 1], F32, kind="ExternalInput").ap()
    wk_d = nc.dram_tensor("wk", [D, DG], BF16, kind="ExternalInput").ap()
    wv_d = nc.dram_tensor("wv", [D + 1, VW], BF16, kind="ExternalInput").ap()
    wo_d = nc.dram_tensor("wo", [DG, D], BF16, kind="ExternalInput").ap()
    if causal:
        bm_d = nc.dram_tensor("bm", [P, P], BF16, kind="ExternalInput").ap()
    else:
        mt_d = nc.dram_tensor("mt", [HG, S, S], BF16, kind="ExternalInput").ap()
    out_d = nc.dram_tensor("out", [D, S], BF16, kind="ExternalOutput").ap()

    EXP = mybir.ActivationFunctionType.Exp

    with tile.TileContext(nc) as tc:
        with (
            tc.tile_pool(name="consts", bufs=1) as consts,
            tc.tile_pool(name="work", bufs=6) as work,
            tc.tile_pool(name="attn", bufs=4) as attnp,
            tc.tile_pool(name="small", bufs=4) as small,
            tc.tile_pool(name="pmm", bufs=2, space="PSUM") as pmm,
            tc.tile_pool(name="pst", bufs=2, space="PSUM") as pst,
            tc.tile_pool(name="patt", bufs=2, space="PSUM") as patt,
        ):
            # ---- persistent SBUF tiles ----
            xts = [
                [
                    consts.tile([P, QB], BF16, tag=f"xt{et}_{sb}", name=f"xt{et}_{sb}")
                    for sb in range(NQB)
                ]
                for et in range(NET)
            ]
            KT = [
                [consts.tile([P, QB], BF16, tag=f"kt{hp}_{sb}", name=f"kt{hp}_{sb}") for sb in range(NQB)]
                for hp in range(2)
            ]
            QT = [
                [consts.tile([P, QB], BF16, tag=f"qt{hp}_{sb}", name=f"qt{hp}_{sb}") for sb in range(NQB)]
                for hp in range(2)
            ]
            V = [
                consts.tile([P, HG * VSEG], BF16, tag=f"v{st}", name=f"v{st}")
                for st in range(NKT)
            ]
            for st in range(NKT):
                # constant ones column at the head of each 128-wide segment
                nc.gpsimd.memset(
                    V[st].rearrange("p (h c) -> p h c", c=VSEG)[:, :, 0:1], 1.0
                )

            # ---- PE warmup burst: ~4us of throwaway matmuls during the
            # initial DMA wait so the HAM clock gate reaches K=8/8 before
            # the first real matmul (cold MMs run at 1.2 instead of 2.4GHz).
            scratch = consts.tile([P, QB], BF16, tag="scratch")
            nc.gpsimd.memset(scratch, 0.0)
            for _ in range(10):
                wps = pmm.tile([P, QB], F32, tag="mm", name="warm")
                nc.tensor.matmul(
                    wps, scratch[:, 0:P], scratch, start=True, stop=True
                )

            # ---- input DMAs, spread across queues: the first projection
            # needs wk+wq+bq+x(sb0); everything else arrives behind it.
            wk_t, wq_t = [], []
            for et in range(NET):
                t = consts.tile([P, DG], BF16, tag=f"wk{et}", name=f"wk{et}")
                nc.sync.dma_start(out=t, in_=wk_d[et * P : (et + 1) * P, :])
                wk_t.append(t)
            # x for sb0 on the gpsimd queue so descriptor issue overlaps
            for et in range(NET):
                nc.gpsimd.dma_start(
                    out=xts[et][0], in_=xT_d[et * P : (et + 1) * P, 0:QB]
                )
            for et in range(NET):
                t = consts.tile([P, DG], BF16, tag=f"wq{et}", name=f"wq{et}")
                nc.scalar.dma_start(out=t, in_=wq_d[et * P : (et + 1) * P, :])
                wq_t.append(t)
            bq_sb = []
            for j in range(2):
                t = consts.tile([P, 1], F32, tag=f"bq{j}", name=f"bq{j}")
                nc.scalar.dma_start(out=t, in_=bq_d[j * P : (j + 1) * P, :])
                bq_sb.append(t)
            wv_t = []
            for et in range(NET):
                t = consts.tile([P, VW], BF16, tag=f"wv{et}", name=f"wv{et}")
                nc.sync.dma_start(out=t, in_=wv_d[et * P : (et + 1) * P, :])
                wv_t.append(t)
            wvb = consts.tile([1, VW], BF16, tag="wvb")
            nc.sync.dma_start(out=wvb, in_=wv_d[D : D + 1, :])
            bvb = consts.tile([P, VW], BF16, tag="bvb")
            nc.gpsimd.partition_broadcast(bvb, wvb)
            if causal:
                bm = consts.tile([P, P], BF16, tag="bm")
                nc.sync.dma_start(out=bm, in_=bm_d)
            for sb in range(1, NQB):
                q_ = [nc.gpsimd, nc.scalar, nc.sync][sb - 1]
                for et in range(NET):
                    q_.dma_start(
                        out=xts[et][sb],
                        in_=xT_d[et * P : (et + 1) * P, sb * QB : (sb + 1) * QB],
                    )
            wo_t = []
            for j in range(2):
                t = consts.tile([P, D], BF16, tag=f"wo{j}", name=f"wo{j}")
                nc.scalar.dma_start(out=t, in_=wo_d[j * P : (j + 1) * P, :])
                wo_t.append(t)

            # ---- projection emission, one s-block at a time ----
            def proj_kq(sb, hp):
                dsl = slice(hp * P, (hp + 1) * P)
                ps = pmm.tile([P, QB], F32, tag="mm")
                for et in range(NET):
                    nc.tensor.matmul(
                        ps, wk_t[et][:, dsl], xts[et][sb],
                        start=(et == 0), stop=(et == NET - 1),
                    )
                nc.vector.tensor_copy(KT[hp][sb], ps)
                ps = pmm.tile([P, QB], F32, tag="mm")
                for et in range(NET):
                    nc.tensor.matmul(
                        ps, wq_t[et][:, dsl], xts[et][sb],
                        start=(et == 0), stop=(et == NET - 1),
                    )
                # fold bq in during the PSUM->SBUF cast
                nc.vector.tensor_scalar_add(QT[hp][sb], ps, bq_sb[hp])

            def proj_v(st):
                sb, loc = st // 4, (st % 4) * P
                ps = pmm.tile([P, VW], F32, tag="mm")
                for et in range(NET):
                    nc.tensor.matmul(
                        ps, xts[et][sb][:, loc : loc + P], wv_t[et],
                        start=(et == 0), stop=(et == NET - 1),
                    )
                nc.vector.tensor_add(
                    V[st].rearrange("p (h c) -> p h c", c=VSEG)[:, :, HD:VSEG],
                    ps.rearrange("p (h c) -> p h c", c=HD),
                    bvb.rearrange("p (h c) -> p h c", c=HD),
                )

            def proj_pieces(sb):
                return [lambda hp=hp: proj_kq(sb, hp) for hp in range(2)] + [
                    lambda st=st: proj_v(st) for st in range(4 * sb, 4 * sb + 4)
                ]

            # emit all of s-block 0's projections up front
            for piece in proj_pieces(0):
                piece()

            def out_proj(qb, attn_t, et):
                qsl = slice(qb * QB, (qb + 1) * QB)
                esl = slice(et * P, (et + 1) * P)
                ops = pmm.tile([P, QB], F32, tag="mm")
                nc.tensor.matmul(
                    ops, wo_t[0][:, esl], attn_t[0], start=True, stop=False
                )
                nc.tensor.matmul(
                    ops, wo_t[1][:, esl], attn_t[1], start=False, stop=True
                )
                ost = work.tile([P, QB], BF16, tag="ost")
                nc.vector.tensor_copy(ost, ops)
                nc.sync.dma_start(out=out_d[esl, qsl], in_=ost)

            # ---- attention, per q-block; the previous q-block's
            # out-projection and the next s-block's projections are
            # interleaved into the unit stream so the PE never head-of-line
            # blocks on exp or on the normalize chain.
            rA, rB = slice(0, HD), slice(HD, 2 * HD)
            deferred = []  # out-proj closures from the previous q-block
            for qb in range(NQB):
                qsl = slice(qb * QB, (qb + 1) * QB)
                nband = 4 * qb + 4  # k-tiles 0..nband-1 are in play
                attn_t = [
                    attnp.tile([P, QB], BF16, tag="attn_t", name=f"attn{i}_{qb}")
                    for i in range(2)
                ]
                pieces = deferred + (
                    list(proj_pieces(qb + 1)) if qb + 1 < NQB else []
                )

                for hp in range(2):
                    hA, hB = 2 * hp, 2 * hp + 1
                    attps = [
                        patt.tile([P, QB], F32, tag="att", name=f"att{qb}_{h}")
                        for h in (hA, hB)
                    ]
                    # software pipeline: AV matmuls for unit u are emitted
                    # after scores+exp of unit u+2 so the PE never
                    # head-of-line blocks on exp.
                    pendings = []

                    def flush_av(kt, exU, woff, w, qo):
                        for h, off, aps in ((hA, woff, attps[0]), (hB, QB, attps[1])):
                            nc.tensor.matmul(
                                aps[:, qo : qo + w],
                                V[kt][:, h * VSEG : (h + 1) * VSEG],
                                exU[:, off : off + w],
                                start=(kt == 0), stop=(kt == nband - 1),
                            )

                    for kt in range(nband):
                        if causal and kt >= 4 * qb:
                            qo = (kt - 4 * qb) * P
                            w = QB - qo
                            tri = True
                        else:
                            qo, w, tri = 0, QB, False
                        woff = QB - w  # head A right-aligned in bank 0
                        stU = pst.tile([P, 2 * QB], F32, tag="stU")
                        sbk, loc = kt // 4, (kt % 4) * P
                        nc.tensor.matmul(
                            stU[:, woff:QB],
                            KT[hp][sbk][rA, loc : loc + P],
                            QT[hp][qb][rA, qo:QB],
                            start=True, stop=True, tile_position=(0, 0),
                        )
                        nc.tensor.matmul(
                            stU[:, QB : QB + w],
                            KT[hp][sbk][rB, loc : loc + P],
                            QT[hp][qb][rB, qo:QB],
                            start=True, stop=True, tile_position=(64, 0),
                        )
                        exU = work.tile([P, 2 * QB], BF16, tag="exU")
                        # scores are q.k / sqrt(64): fold 1/8 into the exp
                        nc.scalar.activation(
                            exU[:, woff : QB + w], stU[:, woff : QB + w],
                            EXP, scale=0.125,
                        )
                        if tri:
                            nc.vector.tensor_mul(
                                exU[:, woff : woff + P], exU[:, woff : woff + P],
                                bm,
                            )
                            nc.vector.tensor_mul(
                                exU[:, QB : QB + P], exU[:, QB : QB + P], bm
                            )
                        elif not causal:
                            for h, off in ((hA, 0), (hB, QB)):
                                mtile = work.tile([P, QB], BF16, tag="mt")
                                nc.sync.dma_start(
                                    out=mtile,
                                    in_=mt_d[h % HG, kt * P : (kt + 1) * P, qsl],
                                )
                                nc.vector.tensor_mul(
                                    exU[:, off : off + QB], exU[:, off : off + QB],
                                    mtile,
                                )
                        pendings.append((kt, exU, woff, w, qo))
                        if len(pendings) > 2:
                            flush_av(*pendings.pop(0))
                        if pieces:
                            pieces.pop(0)()
                    for p_ in pendings:
                        flush_av(*p_)

                    # ---- normalize: partition 0 of attps is sumexp,
                    # partitions 64..127 are sum(exp * v); read PSUM
                    # directly. Recips for both heads emitted before the
                    # broadcasts/muls so the DVE works on head B while head
                    # A's broadcast runs on GpSimd.
                    rcps, rbs = [], []
                    for h, aps in ((hA, attps[0]), (hB, attps[1])):
                        rcp = small.tile([1, QB], F32, tag="rcp")
                        nc.vector.reciprocal_approx_fast(out=rcp, in_=aps[0:1, :])
                        rcps.append(rcp)
                    for rcp in rcps:
                        rb = small.tile([HD, QB], F32, tag="rb")
                        nc.gpsimd.partition_broadcast(rb, rcp)
                        rbs.append(rb)
                    for aps, rb, rsl in (
                        (attps[0], rbs[0], rA),
                        (attps[1], rbs[1], rB),
                    ):
                        nc.vector.tensor_mul(
                            attn_t[hp][rsl, :], aps[64:128, :], rb
                        )

                while pieces:
                    pieces.pop(0)()

                deferred = [
                    lambda qb=qb, attn_t=attn_t, et=et: out_proj(qb, attn_t, et)
                    for et in range(NET)
                ]
            for piece in deferred:
                piece()

    nc.compile()
    return nc


def _get_nc(causal: bool):
    if causal not in _BUILT:
        _BUILT[causal] = _build_nc(causal)
    return _BUILT[causal]


def _band_mask():
    """[128, 128] 0/1 tile: valid iff qi >= ki (leading causal triangle)."""
    ki = np.arange(P)[:, None]
    qi = np.arange(P)[None, :]
    return (qi >= ki).astype(np.float32).astype(NPBF16)


def _prep_core_inputs(x, mask, Wq, bq, Wk, Wv, bv, Wo, causal):
    """Build the 8 per-core input maps (bf16, pre-transposed, biases folded)."""
    bm = _band_mask()
    in_maps = []
    for c in range(8):
        b, hg = c // 2, c % 2
        h0, e0 = hg * HG, hg * DG
        xt = np.ascontiguousarray(x[b].T).astype(NPBF16)
        wq = Wq[e0 : e0 + DG, :].T.astype(NPBF16)
        bqv = np.ascontiguousarray(bq[e0 : e0 + DG][:, None], dtype=np.float32)
        wk = Wk[e0 : e0 + DG, :].T.astype(NPBF16)
        # V weights (values only) with the bias folded in as a final row;
        # the ones columns for the softmax denominator are device-side
        # memsets.
        wv = np.zeros((D + 1, VW), np.float32)
        for h in range(HG):
            eh = e0 + h * HD
            wv[:D, h * HD : (h + 1) * HD] = Wv[eh : eh + HD, :].T
            wv[D, h * HD : (h + 1) * HD] = bv[eh : eh + HD]
        wo = Wo[:, e0 : e0 + DG].T.astype(NPBF16)
        m = {
            "xT": xt,
            "wq": wq,
            "bqv": bqv,
            "wk": wk,
            "wv": wv.astype(NPBF16),
            "wo": wo,
        }
        if causal:
            m["bm"] = bm
        else:
            # transposed multiplicative mask per local head: mt[h, k, q]
            mt = np.ascontiguousarray(
                mask[b, h0 : h0 + HG].transpose(0, 2, 1)
            ).astype(NPBF16)
            m["mt"] = mt
        in_maps.append(m)
    return in_maps


def kernel(**inputs):
    from concourse.bass_utils import run_bass_kernel_spmd

    x = np.asarray(inputs["x"], dtype=np.float32)
    mask = np.asarray(inputs["mask"])
    Wq = np.asarray(inputs["Wq"], dtype=np.float32)
    bq = np.asarray(inputs["bq"], dtype=np.float32)
    Wk = np.asarray(inputs["Wk"], dtype=np.float32)
    Wv = np.asarray(inputs["Wv"], dtype=np.float32)
    bv = np.asarray(inputs["bv"], dtype=np.float32)
    Wo = np.asarray(inputs["Wo"], dtype=np.float32)
    bo = np.asarray(inputs["bo"], dtype=np.float32)
    # bk is softmax-invariant (adds a per-query constant to all logits in a
    # row), so it is deliberately not used.

    causal = bool(
        (mask == np.tril(np.ones((S, S), dtype=bool))[None, None]).all()
    )

    nc = _get_nc(causal)
    in_maps = _prep_core_inputs(x, mask, Wq, bq, Wk, Wv, bv, Wo, causal)
    res = run_bass_kernel_spmd(nc, in_maps, core_ids=list(range(8)))
    out = np.empty((B, S, D), np.float32)
    for b in range(B):
        partial = res.results[2 * b]["out"].astype(np.float32) + res.results[
            2 * b + 1
        ]["out"].astype(np.float32)
        out[b] = partial.T + bo[None, :]
    return out


# revision 11
# speedup vs baseline: 1.0190x; 1.0190x over previous
"""Trainium2 Bass kernel for causal multi-head attention.

Problem: B=4, S=2048, D=512, H=8 heads (head_dim 64), causal mask.
  q = x @ Wq.T + bq ; k = x @ Wk.T + bk ; v = x @ Wv.T + bv
  att = softmax(mask(q k^T / sqrt(64))) @ v ; out = att @ Wo.T + bo

Sharding: 8 cores = (batch b in 0..3) x (head-group hg in 0..1, 4 heads each).
Each core computes its 4 heads' Q/K/V projections, attention, and a partial
out-projection (contribution of its head block). Host sums the two partials
per batch and adds bo. No collectives needed.

v2 design notes (the previous version ran projections and attention as two
serial phases; engine queues execute in program order, so the Scalar engine
sat idle for the first ~40us and the whole kernel was exp-starved):

 - Projections are emitted PER S-BLOCK and interleaved into the attention
   unit stream of the previous q-block, so the PE queue alternates
   projection and attention matmuls and the exp stream starts almost
   immediately.
 - Attention is emitted in "units" of one k-tile x one head-pair. Each unit
   owns a [128, 1024] PSUM tile: head A's scores right-aligned in bank 0
   (cols 512-w..512), head B's in bank 1 (cols 512..512+w) - one contiguous
   exp over exactly 2w columns, no wasted Scalar cycles, and each matmul
   stays inside one PSUM bank. stU is double-buffered so scores(u+1) never
   waits on exp(u).
 - Scores are computed TRANSPOSED (ST[k, q]); exp(ST) is directly the
   moving operand of the attention*V matmul. The softmax denominator falls
   out of the same matmul via a ones-column per head in V. The ones column
   is FIRST in each head's 96-wide V segment (values 32-aligned), so sumexp
   lands on PSUM partition 0
   and 1/sumexp is one reciprocal_approx_fast + one partition_broadcast
   (no 32x32 transpose dance).
 - Head pairs use PE row-groups (rows 0..63 / 64..127, tile_position) so the
   two 64-contraction score matmuls execute concurrently.
 - causal structure exploited exactly: k-tiles above the diagonal skipped,
   band k-tiles only produce their valid q columns, one static [128,128]
   0/1 tile masks the leading triangle.
 - matmul operands bf16, accumulation fp32 in PSUM; output DMA'd as bf16
   (host sums partials in fp32 and adds bo).
 - bk is softmax-invariant (adds a per-query constant) and is not used.

The mask input is verified on the host: if it is exactly the causal mask the
fast path runs; otherwise a generic variant runs that reads a host-prepared
transposed multiplicative mask from DRAM.
"""

import sys

import numpy as np

for _p in ("/opt/trn_rl_repo",):
    if _p not in sys.path:
        sys.path.insert(0, _p)

import ml_dtypes  # noqa: E402

import concourse.bass as bass  # noqa: E402
import concourse.tile as tile  # noqa: E402
from concourse import bacc, mybir  # noqa: E402

B, S, D, H = 4, 2048, 512, 8
HD = D // H  # 64
P = 128
HG = 4  # heads per core
DG = HG * HD  # 256 per-core head dims
QB = 512  # q-block
NQB = S // QB  # 4
NKT = S // P  # 16 k-tiles
NET = D // P  # 4 contraction tiles for projections
VW = HG * HD  # 256: V-projection computes values only; the per-head ones
#   column (softmax denominator) is a memset constant.
VSEG = 128  # per-head V segment: [ones, 63 pad, 64 values] so sumexp lands
#   on partition 0 and values on partitions 64..127 (APs may not cross the
#   partition-64 half boundary mid-pattern, so bases must be 0 or 64)

F32 = mybir.dt.float32
BF16 = mybir.dt.bfloat16
NPBF16 = ml_dtypes.bfloat16

_BUILT = {}


def _build_nc(causal: bool):
    """Build (and bacc-compile) the SPMD single-core program."""
    nc = bacc.Bacc("TRN2", target_bir_lowering=False, debug=False, num_devices=8)

    xT_d = nc.dram_tensor("xT", [D, S], BF16, kind="ExternalInput").ap()
    wq_d = nc.dram_tensor("wq", [D, DG], BF16, kind="ExternalInput").ap()
    bq_d = nc.dram_tensor("bqv", [DG, 1], F32, kind="ExternalInput").ap()
    wk_d = nc.dram_tensor("wk", [D, DG], BF16, kind="ExternalInput").ap()
    wv_d = nc.dram_tensor("wv", [D, VW], BF16, kind="ExternalInput").ap()
    bvb_d = nc.dram_tensor("bvb", [P, VW], BF16, kind="ExternalInput").ap()
    wo_d = nc.dram_tensor("wo", [DG, D], BF16, kind="ExternalInput").ap()
    if causal:
        bm_d = nc.dram_tensor("bm", [P, P], BF16, kind="ExternalInput").ap()
    else:
        mt_d = nc.dram_tensor("mt", [HG, S, S], BF16, kind="ExternalInput").ap()
    out_d = nc.dram_tensor("out", [D, S], BF16, kind="ExternalOutput").ap()

    EXP = mybir.ActivationFunctionType.Exp

    with tile.TileContext(nc) as tc:
        with (
            tc.tile_pool(name="consts", bufs=1) as consts,
            tc.tile_pool(name="work", bufs=6) as work,
            tc.tile_pool(name="attn", bufs=4) as attnp,
            tc.tile_pool(name="small", bufs=4) as small,
            tc.tile_pool(name="pmm", bufs=2, space="PSUM") as pmm,
            tc.tile_pool(name="pst", bufs=2, space="PSUM") as pst,
            tc.tile_pool(name="patt", bufs=2, space="PSUM") as patt,
        ):
            # ---- persistent SBUF tiles ----
            xts = [
                [
                    consts.tile([P, QB], BF16, tag=f"xt{et}_{sb}", name=f"xt{et}_{sb}")
                    for sb in range(NQB)
                ]
                for et in range(NET)
            ]
            KT = [
                [consts.tile([P, QB], BF16, tag=f"kt{hp}_{sb}", name=f"kt{hp}_{sb}") for sb in range(NQB)]
                for hp in range(2)
            ]
            QT = [
                [consts.tile([P, QB], BF16, tag=f"qt{hp}_{sb}", name=f"qt{hp}_{sb}") for sb in range(NQB)]
                for hp in range(2)
            ]
            V = [
                consts.tile([P, HG * VSEG], BF16, tag=f"v{st}", name=f"v{st}")
                for st in range(NKT)
            ]
            for st in range(NKT):
                # constant ones column at the head of each 128-wide segment
                nc.vector.memset(
                    V[st].rearrange("p (h c) -> p h c", c=VSEG)[:, :, 0:1], 1.0
                )

            # ---- PE warmup burst: ~4us of throwaway matmuls during the
            # initial DMA wait so the HAM clock gate reaches K=8/8 before
            # the first real matmul (cold MMs run at 1.2 instead of 2.4GHz).
            scratch = consts.tile([P, QB], BF16, tag="scratch")
            nc.vector.memset(scratch, 0.0)
            for _ in range(16):
                wps = pmm.tile([P, QB], F32, tag="mm", name="warm")
                nc.tensor.matmul(
                    wps, scratch[:, 0:P], scratch, start=True, stop=True
                )

            # ---- input DMAs, spread across queues. The first exp needs
            # wk+wq+bq+x(sb0): interleave x(sb0)/wk on the sync queue,
            # wq/bq on gpsimd, and everything consumed later (wv, bvb, bm,
            # wo) on the scalar queue, which is idle until the exp stream
            # starts at ~13us.
            wk_t, wq_t = [], []
            for et in range(NET):
                nc.sync.dma_start(
                    out=xts[et][0], in_=xT_d[et * P : (et + 1) * P, 0:QB]
                )
                t = consts.tile([P, DG], BF16, tag=f"wk{et}", name=f"wk{et}")
                nc.sync.dma_start(out=t, in_=wk_d[et * P : (et + 1) * P, :])
                wk_t.append(t)
            for et in range(NET):
                t = consts.tile([P, DG], BF16, tag=f"wq{et}", name=f"wq{et}")
                nc.gpsimd.dma_start(out=t, in_=wq_d[et * P : (et + 1) * P, :])
                wq_t.append(t)
            bq_sb = []
            for j in range(2):
                t = consts.tile([P, 1], F32, tag=f"bq{j}", name=f"bq{j}")
                nc.gpsimd.dma_start(out=t, in_=bq_d[j * P : (j + 1) * P, :])
                bq_sb.append(t)
            wv_t = []
            for et in range(NET):
                t = consts.tile([P, VW], BF16, tag=f"wv{et}", name=f"wv{et}")
                nc.scalar.dma_start(out=t, in_=wv_d[et * P : (et + 1) * P, :])
                wv_t.append(t)
            bvb = consts.tile([P, VW], BF16, tag="bvb")
            nc.scalar.dma_start(out=bvb, in_=bvb_d)
            if causal:
                bm = consts.tile([P, P], BF16, tag="bm")
                nc.scalar.dma_start(out=bm, in_=bm_d)
            wo_t = []
            for j in range(2):
                t = consts.tile([P, D], BF16, tag=f"wo{j}", name=f"wo{j}")
                nc.scalar.dma_start(out=t, in_=wo_d[j * P : (j + 1) * P, :])
                wo_t.append(t)
            for sb in range(1, NQB):
                q_ = [nc.gpsimd, nc.scalar, nc.sync][sb - 1]
                for et in range(NET):
                    q_.dma_start(
                        out=xts[et][sb],
                        in_=xT_d[et * P : (et + 1) * P, sb * QB : (sb + 1) * QB],
                    )

            # ---- projection emission, one s-block at a time ----
            def proj_kq(sb, hp):
                dsl = slice(hp * P, (hp + 1) * P)
                ps = pmm.tile([P, QB], F32, tag="mm")
                for et in range(NET):
                    nc.tensor.matmul(
                        ps, wk_t[et][:, dsl], xts[et][sb],
                        start=(et == 0), stop=(et == NET - 1),
                    )
                nc.vector.tensor_copy(KT[hp][sb], ps)
                ps = pmm.tile([P, QB], F32, tag="mm")
                for et in range(NET):
                    nc.tensor.matmul(
                        ps, wq_t[et][:, dsl], xts[et][sb],
                        start=(et == 0), stop=(et == NET - 1),
                    )
                # fold bq in during the PSUM->SBUF cast
                nc.vector.tensor_scalar_add(QT[hp][sb], ps, bq_sb[hp])

            def proj_v(st):
                sb, loc = st // 4, (st % 4) * P
                ps = pmm.tile([P, VW], F32, tag="mm")
                for et in range(NET):
                    nc.tensor.matmul(
                        ps, xts[et][sb][:, loc : loc + P], wv_t[et],
                        start=(et == 0), stop=(et == NET - 1),
                    )
                nc.vector.tensor_add(
                    V[st].rearrange("p (h c) -> p h c", c=VSEG)[:, :, HD:VSEG],
                    ps.rearrange("p (h c) -> p h c", c=HD),
                    bvb.rearrange("p (h c) -> p h c", c=HD),
                )

            def proj_pieces(sb):
                return [lambda hp=hp: proj_kq(sb, hp) for hp in range(2)] + [
                    lambda st=st: proj_v(st) for st in range(4 * sb, 4 * sb + 4)
                ]

            # emit all of s-block 0's projections up front
            for piece in proj_pieces(0):
                piece()

            def out_proj(qb, attn_t, et):
                qsl = slice(qb * QB, (qb + 1) * QB)
                esl = slice(et * P, (et + 1) * P)
                ops = pmm.tile([P, QB], F32, tag="mm")
                nc.tensor.matmul(
                    ops, wo_t[0][:, esl], attn_t[0], start=True, stop=False
                )
                nc.tensor.matmul(
                    ops, wo_t[1][:, esl], attn_t[1], start=False, stop=True
                )
                ost = work.tile([P, QB], BF16, tag="ost")
                nc.vector.tensor_copy(ost, ops)
                nc.sync.dma_start(out=out_d[esl, qsl], in_=ost)

            # ---- attention, per q-block; the previous q-block's
            # out-projection and the next s-block's projections are
            # interleaved into the unit stream so the PE never head-of-line
            # blocks on exp or on the normalize chain.
            rA, rB = slice(0, HD), slice(HD, 2 * HD)
            deferred = []  # out-proj closures from the previous q-block
            for qb in range(NQB):
                qsl = slice(qb * QB, (qb + 1) * QB)
                nband = 4 * qb + 4  # k-tiles 0..nband-1 are in play
                attn_t = [
                    attnp.tile([P, QB], BF16, tag="attn_t", name=f"attn{i}_{qb}")
                    for i in range(2)
                ]
                pieces = deferred + (
                    list(proj_pieces(qb + 1)) if qb + 1 < NQB else []
                )

                for hp in range(2):
                    hA, hB = 2 * hp, 2 * hp + 1
                    attps = [
                        patt.tile([P, QB], F32, tag="att", name=f"att{qb}_{h}")
                        for h in (hA, hB)
                    ]
                    # software pipeline: AV matmuls for unit u are emitted
                    # after scores+exp of unit u+2 so the PE never
                    # head-of-line blocks on exp.
                    pendings = []

                    def flush_av(kt, exU, woff, w, qo):
                        for h, off, aps in ((hA, woff, attps[0]), (hB, QB, attps[1])):
                            nc.tensor.matmul(
                                aps[:, qo : qo + w],
                                V[kt][:, h * VSEG : (h + 1) * VSEG],
                                exU[:, off : off + w],
                                start=(kt == 0), stop=(kt == nband - 1),
                            )

                    for kt in range(nband):
                        if causal and kt >= 4 * qb:
                            qo = (kt - 4 * qb) * P
                            w = QB - qo
                            tri = True
                        else:
                            qo, w, tri = 0, QB, False
                        woff = QB - w  # head A right-aligned in bank 0
                        stU = pst.tile([P, 2 * QB], F32, tag="stU")
                        sbk, loc = kt // 4, (kt % 4) * P
                        nc.tensor.matmul(
                            stU[:, woff:QB],
                            KT[hp][sbk][rA, loc : loc + P],
                            QT[hp][qb][rA, qo:QB],
                            start=True, stop=True, tile_position=(0, 0),
                        )
                        nc.tensor.matmul(
                            stU[:, QB : QB + w],
                            KT[hp][sbk][rB, loc : loc + P],
                            QT[hp][qb][rB, qo:QB],
                            start=True, stop=True, tile_position=(64, 0),
                        )
                        exU = work.tile([P, 2 * QB], BF16, tag="exU")
                        # scores are q.k / sqrt(64): fold 1/8 into the exp
                        nc.scalar.activation(
                            exU[:, woff : QB + w], stU[:, woff : QB + w],
                            EXP, scale=0.125,
                        )
                        if tri:
                            nc.vector.tensor_mul(
                                exU[:, woff : woff + P], exU[:, woff : woff + P],
                                bm,
                            )
                            nc.vector.tensor_mul(
                                exU[:, QB : QB + P], exU[:, QB : QB + P], bm
                            )
                        elif not causal:
                            for h, off in ((hA, 0), (hB, QB)):
                                mtile = work.tile([P, QB], BF16, tag="mt")
                                nc.sync.dma_start(
                                    out=mtile,
                                    in_=mt_d[h % HG, kt * P : (kt + 1) * P, qsl],
                                )
                                nc.vector.tensor_mul(
                                    exU[:, off : off + QB], exU[:, off : off + QB],
                                    mtile,
                                )
                        pendings.append((kt, exU, woff, w, qo))
                        if len(pendings) > 2:
                            flush_av(*pendings.pop(0))
                        if pieces:
                            pieces.pop(0)()
                    for p_ in pendings:
                        flush_av(*p_)

                    # ---- normalize: partition 0 of attps is sumexp,
                    # partitions 64..127 are sum(exp * v); read PSUM
                    # directly. Recips for both heads emitted before the
                    # broadcasts/muls so the DVE works on head B while head
                    # A's broadcast runs on GpSimd.
                    rcps, rbs = [], []
                    for h, aps in ((hA, attps[0]), (hB, attps[1])):
                        rcp = small.tile([1, QB], F32, tag="rcp")
                        nc.vector.reciprocal_approx_fast(out=rcp, in_=aps[0:1, :])
                        rcps.append(rcp)
                    for rcp in rcps:
                        rb = small.tile([HD, QB], F32, tag="rb")
                        nc.gpsimd.partition_broadcast(rb, rcp)
                        rbs.append(rb)
                    for aps, rb, rsl in (
                        (attps[0], rbs[0], rA),
                        (attps[1], rbs[1], rB),
                    ):
                        nc.vector.tensor_mul(
                            attn_t[hp][rsl, :], aps[64:128, :], rb
                        )

                while pieces:
                    pieces.pop(0)()

                deferred = [
                    lambda qb=qb, attn_t=attn_t, et=et: out_proj(qb, attn_t, et)
                    for et in range(NET)
                ]
            for piece in deferred:
                piece()

    nc.compile()
    return nc


def _get_nc(causal: bool):
    if causal not in _BUILT:
        _BUILT[causal] = _build_nc(causal)
    return _BUILT[causal]


def _band_mask():
    """[128, 128] 0/1 tile: valid iff qi >= ki (leading causal triangle)."""
    ki = np.arange(P)[:, None]
    qi = np.arange(P)[None, :]
    return (qi >= ki).astype(np.float32).astype(NPBF16)


def _prep_core_inputs(x, mask, Wq, bq, Wk, Wv, bv, Wo, causal):
    """Build the 8 per-core input maps (bf16, pre-transposed, biases folded)."""
    bm = _band_mask()
    in_maps = []
    for c in range(8):
        b, hg = c // 2, c % 2
        h0, e0 = hg * HG, hg * DG
        xt = np.ascontiguousarray(x[b].T).astype(NPBF16)
        wq = Wq[e0 : e0 + DG, :].T.astype(NPBF16)
        bqv = np.ascontiguousarray(bq[e0 : e0 + DG][:, None], dtype=np.float32)
        wk = Wk[e0 : e0 + DG, :].T.astype(NPBF16)
        # V weights (values only) with the bias folded in as a final row;
        # the ones columns for the softmax denominator are device-side
        # memsets.
        wv = np.ascontiguousarray(Wv[e0 : e0 + DG, :].T)
        bvb = np.broadcast_to(bv[e0 : e0 + DG][None, :], (P, VW))
        wo = Wo[:, e0 : e0 + DG].T.astype(NPBF16)
        m = {
            "xT": xt,
            "wq": wq,
            "bqv": bqv,
            "wk": wk,
            "wv": wv.astype(NPBF16),
            "bvb": np.ascontiguousarray(bvb).astype(NPBF16),
            "wo": wo,
        }
        if causal:
            m["bm"] = bm
        else:
            # transposed multiplicative mask per local head: mt[h, k, q]
            mt = np.ascontiguousarray(
                mask[b, h0 : h0 + HG].transpose(0, 2, 1)
            ).astype(NPBF16)
            m["mt"] = mt
        in_maps.append(m)
    return in_maps


def kernel(**inputs):
    from concourse.bass_utils import run_bass_kernel_spmd

    x = np.asarray(inputs["x"], dtype=np.float32)
    mask = np.asarray(inputs["mask"])
    Wq = np.asarray(inputs["Wq"], dtype=np.float32)
    bq = np.asarray(inputs["bq"], dtype=np.float32)
    Wk = np.asarray(inputs["Wk"], dtype=np.float32)
    Wv = np.asarray(inputs["Wv"], dtype=np.float32)
    bv = np.asarray(inputs["bv"], dtype=np.float32)
    Wo = np.asarray(inputs["Wo"], dtype=np.float32)
    bo = np.asarray(inputs["bo"], dtype=np.float32)
    # bk is softmax-invariant (adds a per-query constant to all logits in a
    # row), so it is deliberately not used.

    causal = bool(
        (mask == np.tril(np.ones((S, S), dtype=bool))[None, None]).all()
    )

    nc = _get_nc(causal)
    in_maps = _prep_core_inputs(x, mask, Wq, bq, Wk, Wv, bv, Wo, causal)
    res = run_bass_kernel_spmd(nc, in_maps, core_ids=list(range(8)))
    out = np.empty((B, S, D), np.float32)
    for b in range(B):
        partial = res.results[2 * b]["out"].astype(np.float32) + res.results[
            2 * b + 1
        ]["out"].astype(np.float32)
        out[b] = partial.T + bo[None, :]
    return out


# revision 13
# speedup vs baseline: 1.0706x; 1.0506x over previous
"""Trainium2 Bass kernel for causal multi-head attention.

Problem: B=4, S=2048, D=512, H=8 heads (head_dim 64), causal mask.
  q = x @ Wq.T + bq ; k = x @ Wk.T + bk ; v = x @ Wv.T + bv
  att = softmax(mask(q k^T / sqrt(64))) @ v ; out = att @ Wo.T + bo

Sharding: 8 cores = (batch b in 0..3) x (head-group hg in 0..1, 4 heads each).
Each core computes its 4 heads' Q/K/V projections, attention, and a partial
out-projection (contribution of its head block). Host sums the two partials
per batch and adds bo. No collectives needed.

v2 design notes (the previous version ran projections and attention as two
serial phases; engine queues execute in program order, so the Scalar engine
sat idle for the first ~40us and the whole kernel was exp-starved):

 - Projections are emitted PER S-BLOCK and interleaved into the attention
   unit stream of the previous q-block, so the PE queue alternates
   projection and attention matmuls and the exp stream starts almost
   immediately.
 - Attention is emitted in "units" of one k-tile x one head-pair. Each unit
   owns a [128, 1024] PSUM tile: head A's scores right-aligned in bank 0
   (cols 512-w..512), head B's in bank 1 (cols 512..512+w) - one contiguous
   exp over exactly 2w columns, no wasted Scalar cycles, and each matmul
   stays inside one PSUM bank. stU is double-buffered so scores(u+1) never
   waits on exp(u).
 - Scores are computed TRANSPOSED (ST[k, q]); exp(ST) is directly the
   moving operand of the attention*V matmul. The softmax denominator falls
   out of the same matmul via a ones-column per head in V. The ones column
   is FIRST in each head's 96-wide V segment (values 32-aligned), so sumexp
   lands on PSUM partition 0
   and 1/sumexp is one reciprocal_approx_fast + one partition_broadcast
   (no 32x32 transpose dance).
 - Head pairs use PE row-groups (rows 0..63 / 64..127, tile_position) so the
   two 64-contraction score matmuls execute concurrently.
 - causal structure exploited exactly: k-tiles above the diagonal skipped,
   band k-tiles only produce their valid q columns, one static [128,128]
   0/1 tile masks the leading triangle.
 - matmul operands bf16, accumulation fp32 in PSUM; output DMA'd as bf16
   (host sums partials in fp32 and adds bo).
 - bk is softmax-invariant (adds a per-query constant) and is not used.

The mask input is verified on the host: if it is exactly the causal mask the
fast path runs; otherwise a generic variant runs that reads a host-prepared
transposed multiplicative mask from DRAM.
"""

import sys

import numpy as np

for _p in ("/opt/trn_rl_repo",):
    if _p not in sys.path:
        sys.path.insert(0, _p)

import ml_dtypes  # noqa: E402

import concourse.bass as bass  # noqa: E402
import concourse.tile as tile  # noqa: E402
from concourse import bacc, mybir  # noqa: E402

B, S, D, H = 4, 2048, 512, 8
HD = D // H  # 64
P = 128
HG = 4  # heads per core
DG = HG * HD  # 256 per-core head dims
QB = 512  # q-block
NQB = S // QB  # 4
NKT = S // P  # 16 k-tiles
NET = D // P  # 4 contraction tiles for projections
VW = HG * HD  # 256: V-projection computes values only; the per-head ones
#   column (softmax denominator) is a memset constant.
VSEG = 128  # per-head V segment: [ones, 63 pad, 64 values] so sumexp lands
#   on partition 0 and values on partitions 64..127 (APs may not cross the
#   partition-64 half boundary mid-pattern, so bases must be 0 or 64)

F32 = mybir.dt.float32
BF16 = mybir.dt.bfloat16
NPBF16 = ml_dtypes.bfloat16

_BUILT = {}


def _build_nc(causal: bool):
    """Build (and bacc-compile) the SPMD single-core program."""
    nc = bacc.Bacc("TRN2", target_bir_lowering=False, debug=False, num_devices=8)

    xT_d = nc.dram_tensor("xT", [D, S], BF16, kind="ExternalInput").ap()
    wq_d = nc.dram_tensor("wq", [D, DG], BF16, kind="ExternalInput").ap()
    bq_d = nc.dram_tensor("bqv", [DG, 1], F32, kind="ExternalInput").ap()
    wk_d = nc.dram_tensor("wk", [D, DG], BF16, kind="ExternalInput").ap()
    wv_d = nc.dram_tensor("wv", [D, VW], BF16, kind="ExternalInput").ap()
    bvb_d = nc.dram_tensor("bvb", [P, VW], BF16, kind="ExternalInput").ap()
    wo_d = nc.dram_tensor("wo", [DG, D], BF16, kind="ExternalInput").ap()
    if causal:
        bm_d = nc.dram_tensor("bm", [P, P], BF16, kind="ExternalInput").ap()
    else:
        mt_d = nc.dram_tensor("mt", [HG, S, S], BF16, kind="ExternalInput").ap()
    out_d = nc.dram_tensor("out", [D, S], BF16, kind="ExternalOutput").ap()

    EXP = mybir.ActivationFunctionType.Exp

    with tile.TileContext(nc) as tc:
        with (
            tc.tile_pool(name="consts", bufs=1) as consts,
            tc.tile_pool(name="work", bufs=6) as work,
            tc.tile_pool(name="attn", bufs=4) as attnp,
            tc.tile_pool(name="small", bufs=4) as small,
            tc.tile_pool(name="pmm", bufs=2, space="PSUM") as pmm,
            tc.tile_pool(name="pst", bufs=2, space="PSUM") as pst,
            tc.tile_pool(name="patt", bufs=2, space="PSUM") as patt,
        ):
            # ---- persistent SBUF tiles ----
            xts = [
                [
                    consts.tile([P, QB], BF16, tag=f"xt{et}_{sb}", name=f"xt{et}_{sb}")
                    for sb in range(NQB)
                ]
                for et in range(NET)
            ]
            KT = [
                [consts.tile([P, QB], BF16, tag=f"kt{hp}_{sb}", name=f"kt{hp}_{sb}") for sb in range(NQB)]
                for hp in range(2)
            ]
            QT = [
                [consts.tile([P, QB], BF16, tag=f"qt{hp}_{sb}", name=f"qt{hp}_{sb}") for sb in range(NQB)]
                for hp in range(2)
            ]
            V = [
                consts.tile([P, HG * VSEG], BF16, tag=f"v{st}", name=f"v{st}")
                for st in range(NKT)
            ]
            for st in range(NKT):
                # constant ones column at the head of each 128-wide segment
                nc.vector.memset(
                    V[st].rearrange("p (h c) -> p h c", c=VSEG)[:, :, 0:1], 1.0
                )

            # ---- input DMAs, spread across queues. The first exp needs
            # wk+wq+bq+x(sb0): interleave x(sb0)/wk on the sync queue,
            # wq/bq on gpsimd, and everything consumed later (wv, bvb, bm,
            # wo) on the scalar queue, which is idle until the exp stream
            # starts at ~13us.
            wk_t, wq_t = [], []
            for et in range(NET):
                nc.sync.dma_start(
                    out=xts[et][0], in_=xT_d[et * P : (et + 1) * P, 0:QB]
                )
                t = consts.tile([P, DG], BF16, tag=f"wk{et}", name=f"wk{et}")
                nc.sync.dma_start(out=t, in_=wk_d[et * P : (et + 1) * P, :])
                wk_t.append(t)
            for et in range(NET):
                t = consts.tile([P, DG], BF16, tag=f"wq{et}", name=f"wq{et}")
                nc.gpsimd.dma_start(out=t, in_=wq_d[et * P : (et + 1) * P, :])
                wq_t.append(t)
            bq_sb = []
            for j in range(2):
                t = consts.tile([P, 1], F32, tag=f"bq{j}", name=f"bq{j}")
                nc.gpsimd.dma_start(out=t, in_=bq_d[j * P : (j + 1) * P, :])
                bq_sb.append(t)
            wv_t = []
            for et in range(NET):
                t = consts.tile([P, VW], BF16, tag=f"wv{et}", name=f"wv{et}")
                nc.scalar.dma_start(out=t, in_=wv_d[et * P : (et + 1) * P, :])
                wv_t.append(t)
            bvb = consts.tile([P, VW], BF16, tag="bvb")
            nc.scalar.dma_start(out=bvb, in_=bvb_d)
            if causal:
                bm = consts.tile([P, P], BF16, tag="bm")
                nc.scalar.dma_start(out=bm, in_=bm_d)
            wo_t = []
            for j in range(2):
                t = consts.tile([P, D], BF16, tag=f"wo{j}", name=f"wo{j}")
                nc.scalar.dma_start(out=t, in_=wo_d[j * P : (j + 1) * P, :])
                wo_t.append(t)
            for sb in range(1, NQB):
                q_ = [nc.gpsimd, nc.scalar, nc.sync][sb - 1]
                for et in range(NET):
                    q_.dma_start(
                        out=xts[et][sb],
                        in_=xT_d[et * P : (et + 1) * P, sb * QB : (sb + 1) * QB],
                    )

            # ---- projection emission, in fine-grained pieces (~2 matmuls
            # each) that the attention loop sprinkles between units so the
            # PE load stays smooth and the exp stream never starves.
            def proj_kq(sb, hp, wt, dst, bias, state, half):
                dsl = slice(hp * P, (hp + 1) * P)
                if half == 0:
                    state["ps"] = pmm.tile([P, QB], F32, tag="mm", name="kq")
                    for et in (0, 1):
                        nc.tensor.matmul(
                            state["ps"], wt[et][:, dsl], xts[et][sb],
                            start=(et == 0), stop=False,
                        )
                else:
                    ps = state.pop("ps")
                    for et in (2, 3):
                        nc.tensor.matmul(
                            ps, wt[et][:, dsl], xts[et][sb],
                            start=False, stop=(et == 3),
                        )
                    if bias is None:
                        nc.vector.tensor_copy(dst, ps)
                    else:
                        # fold bq in during the PSUM->SBUF cast
                        nc.vector.tensor_scalar_add(dst, ps, bias)

            def proj_v(st):
                sb, loc = st // 4, (st % 4) * P
                ps = pmm.tile([P, VW], F32, tag="mm")
                for et in range(NET):
                    nc.tensor.matmul(
                        ps, xts[et][sb][:, loc : loc + P], wv_t[et],
                        start=(et == 0), stop=(et == NET - 1),
                    )
                nc.vector.tensor_add(
                    V[st].rearrange("p (h c) -> p h c", c=VSEG)[:, :, HD:VSEG],
                    ps.rearrange("p (h c) -> p h c", c=HD),
                    bvb.rearrange("p (h c) -> p h c", c=HD),
                )

            def kq_pieces(sb, hp):
                st_k, st_q = {}, {}
                return [
                    lambda: proj_kq(sb, hp, wk_t, KT[hp][sb], None, st_k, 0),
                    lambda: proj_kq(sb, hp, wk_t, KT[hp][sb], None, st_k, 1),
                    lambda: proj_kq(sb, hp, wq_t, QT[hp][sb], bq_sb[hp], st_q, 0),
                    lambda: proj_kq(sb, hp, wq_t, QT[hp][sb], bq_sb[hp], st_q, 1),
                ]

            def proj_pieces(sb):
                return (
                    kq_pieces(sb, 0)
                    + kq_pieces(sb, 1)
                    + [lambda st=st: proj_v(st) for st in range(4 * sb, 4 * sb + 4)]
                )

            # up front: only what the very first score matmul needs (K/Q of
            # s-block 0 for head-pair 0); the rest of s-block 0 is the head
            # of q-block 0's piece stream.
            first = proj_pieces(0)
            for piece in first[:4]:
                piece()
            leftover = first[4:]

            def out_proj(qb, attn_t, et):
                qsl = slice(qb * QB, (qb + 1) * QB)
                esl = slice(et * P, (et + 1) * P)
                ops = pmm.tile([P, QB], F32, tag="mm")
                nc.tensor.matmul(
                    ops, wo_t[0][:, esl], attn_t[0], start=True, stop=False
                )
                nc.tensor.matmul(
                    ops, wo_t[1][:, esl], attn_t[1], start=False, stop=True
                )
                ost = work.tile([P, QB], BF16, tag="ost")
                nc.vector.tensor_copy(ost, ops)
                nc.sync.dma_start(out=out_d[esl, qsl], in_=ost)

            # ---- attention, per q-block; the previous q-block's
            # out-projection and the next s-block's projections are
            # interleaved into the unit stream so the PE never head-of-line
            # blocks on exp or on the normalize chain.
            rA, rB = slice(0, HD), slice(HD, 2 * HD)
            deferred = []  # out-proj closures from the previous q-block
            for qb in range(NQB):
                qsl = slice(qb * QB, (qb + 1) * QB)
                nband = 4 * qb + 4  # k-tiles 0..nband-1 are in play
                attn_t = [
                    attnp.tile([P, QB], BF16, tag="attn_t", name=f"attn{i}_{qb}")
                    for i in range(2)
                ]
                pieces = leftover + deferred + (
                    list(proj_pieces(qb + 1)) if qb + 1 < NQB else []
                )
                leftover = []

                for hp in range(2):
                    hA, hB = 2 * hp, 2 * hp + 1
                    attps = [
                        patt.tile([P, QB], F32, tag="att", name=f"att{qb}_{h}")
                        for h in (hA, hB)
                    ]
                    # software pipeline: AV matmuls for unit u are emitted
                    # after scores+exp of unit u+2 so the PE never
                    # head-of-line blocks on exp.
                    pendings = []

                    def flush_av(kt, exU, woff, w, qo):
                        for h, off, aps in ((hA, woff, attps[0]), (hB, QB, attps[1])):
                            nc.tensor.matmul(
                                aps[:, qo : qo + w],
                                V[kt][:, h * VSEG : (h + 1) * VSEG],
                                exU[:, off : off + w],
                                start=(kt == 0), stop=(kt == nband - 1),
                            )

                    for kt in range(nband):
                        if causal and kt >= 4 * qb:
                            qo = (kt - 4 * qb) * P
                            w = QB - qo
                            tri = True
                        else:
                            qo, w, tri = 0, QB, False
                        woff = QB - w  # head A right-aligned in bank 0
                        stU = pst.tile([P, 2 * QB], F32, tag="stU")
                        sbk, loc = kt // 4, (kt % 4) * P
                        nc.tensor.matmul(
                            stU[:, woff:QB],
                            KT[hp][sbk][rA, loc : loc + P],
                            QT[hp][qb][rA, qo:QB],
                            start=True, stop=True, tile_position=(0, 0),
                        )
                        nc.tensor.matmul(
                            stU[:, QB : QB + w],
                            KT[hp][sbk][rB, loc : loc + P],
                            QT[hp][qb][rB, qo:QB],
                            start=True, stop=True, tile_position=(64, 0),
                        )
                        exU = work.tile([P, 2 * QB], BF16, tag="exU")
                        # scores are q.k / sqrt(64): fold 1/8 into the exp
                        nc.scalar.activation(
                            exU[:, woff : QB + w], stU[:, woff : QB + w],
                            EXP, scale=0.125,
                        )
                        if tri:
                            nc.vector.tensor_mul(
                                exU[:, woff : woff + P], exU[:, woff : woff + P],
                                bm,
                            )
                            nc.vector.tensor_mul(
                                exU[:, QB : QB + P], exU[:, QB : QB + P], bm
                            )
                        elif not causal:
                            for h, off in ((hA, 0), (hB, QB)):
                                mtile = work.tile([P, QB], BF16, tag="mt")
                                nc.sync.dma_start(
                                    out=mtile,
                                    in_=mt_d[h % HG, kt * P : (kt + 1) * P, qsl],
                                )
                                nc.vector.tensor_mul(
                                    exU[:, off : off + QB], exU[:, off : off + QB],
                                    mtile,
                                )
                        pendings.append((kt, exU, woff, w, qo))
                        if len(pendings) > 2:
                            flush_av(*pendings.pop(0))
                        # adaptive: spread remaining pieces over remaining
                        # units of this q-block
                        units_left = 2 * nband - (hp * nband + kt + 1) + 1
                        npop = -(-len(pieces) // units_left) if pieces else 0
                        for _ in range(min(npop, len(pieces))):
                            pieces.pop(0)()
                    for p_ in pendings:
                        flush_av(*p_)

                    # ---- normalize: partition 0 of attps is sumexp,
                    # partitions 64..127 are sum(exp * v); read PSUM
                    # directly. Recips for both heads emitted before the
                    # broadcasts/muls so the DVE works on head B while head
                    # A's broadcast runs on GpSimd.
                    rcps, rbs = [], []
                    for h, aps in ((hA, attps[0]), (hB, attps[1])):
                        rcp = small.tile([1, QB], F32, tag="rcp")
                        nc.vector.reciprocal_approx_fast(out=rcp, in_=aps[0:1, :])
                        rcps.append(rcp)
                    for rcp in rcps:
                        rb = small.tile([HD, QB], F32, tag="rb")
                        nc.gpsimd.partition_broadcast(rb, rcp)
                        rbs.append(rb)
                    for aps, rb, rsl in (
                        (attps[0], rbs[0], rA),
                        (attps[1], rbs[1], rB),
                    ):
                        nc.vector.tensor_mul(
                            attn_t[hp][rsl, :], aps[64:128, :], rb
                        )

                while pieces:
                    pieces.pop(0)()

                deferred = [
                    lambda qb=qb, attn_t=attn_t, et=et: out_proj(qb, attn_t, et)
                    for et in range(NET)
                ]
            for piece in deferred:
                piece()

    nc.compile()
    return nc


def _get_nc(causal: bool):
    if causal not in _BUILT:
        _BUILT[causal] = _build_nc(causal)
    return _BUILT[causal]


def _band_mask():
    """[128, 128] 0/1 tile: valid iff qi >= ki (leading causal triangle)."""
    ki = np.arange(P)[:, None]
    qi = np.arange(P)[None, :]
    return (qi >= ki).astype(np.float32).astype(NPBF16)


def _prep_core_inputs(x, mask, Wq, bq, Wk, Wv, bv, Wo, causal):
    """Build the 8 per-core input maps (bf16, pre-transposed, biases folded)."""
    bm = _band_mask()
    in_maps = []
    for c in range(8):
        b, hg = c // 2, c % 2
        h0, e0 = hg * HG, hg * DG
        xt = np.ascontiguousarray(x[b].T).astype(NPBF16)
        wq = Wq[e0 : e0 + DG, :].T.astype(NPBF16)
        bqv = np.ascontiguousarray(bq[e0 : e0 + DG][:, None], dtype=np.float32)
        wk = Wk[e0 : e0 + DG, :].T.astype(NPBF16)
        # V weights (values only) with the bias folded in as a final row;
        # the ones columns for the softmax denominator are device-side
        # memsets.
        wv = np.ascontiguousarray(Wv[e0 : e0 + DG, :].T)
        bvb = np.broadcast_to(bv[e0 : e0 + DG][None, :], (P, VW))
        wo = Wo[:, e0 : e0 + DG].T.astype(NPBF16)
        m = {
            "xT": xt,
            "wq": wq,
            "bqv": bqv,
            "wk": wk,
            "wv": wv.astype(NPBF16),
            "bvb": np.ascontiguousarray(bvb).astype(NPBF16),
            "wo": wo,
        }
        if causal:
            m["bm"] = bm
        else:
            # transposed multiplicative mask per local head: mt[h, k, q]
            mt = np.ascontiguousarray(
                mask[b, h0 : h0 + HG].transpose(0, 2, 1)
            ).astype(NPBF16)
            m["mt"] = mt
        in_maps.append(m)
    return in_maps


def kernel(**inputs):
    from concourse.bass_utils import run_bass_kernel_spmd

    x = np.asarray(inputs["x"], dtype=np.float32)
    mask = np.asarray(inputs["mask"])
    Wq = np.asarray(inputs["Wq"], dtype=np.float32)
    bq = np.asarray(inputs["bq"], dtype=np.float32)
    Wk = np.asarray(inputs["Wk"], dtype=np.float32)
    Wv = np.asarray(inputs["Wv"], dtype=np.float32)
    bv = np.asarray(inputs["bv"], dtype=np.float32)
    Wo = np.asarray(inputs["Wo"], dtype=np.float32)
    bo = np.asarray(inputs["bo"], dtype=np.float32)
    # bk is softmax-invariant (adds a per-query constant to all logits in a
    # row), so it is deliberately not used.

    causal = bool(
        (mask == np.tril(np.ones((S, S), dtype=bool))[None, None]).all()
    )

    nc = _get_nc(causal)
    in_maps = _prep_core_inputs(x, mask, Wq, bq, Wk, Wv, bv, Wo, causal)
    res = run_bass_kernel_spmd(nc, in_maps, core_ids=list(range(8)))
    out = np.empty((B, S, D), np.float32)
    for b in range(B):
        partial = res.results[2 * b]["out"].astype(np.float32) + res.results[
            2 * b + 1
        ]["out"].astype(np.float32)
        out[b] = partial.T + bo[None, :]
    return out
